# revision 1
# baseline (speedup 1.0000x reference)
"""Trainium2 Bass kernel: forward kinematics of a 32-link serial chain.

Reference computes, per batch element b (262144 of them), a sequential scan
over 32 links maintaining a world rotation R and translation t, emitting
per-link [t, quat(R)] (quat in copysign/canonical form, w >= 0).

Device algorithm (all elementwise over the batch):
  - State is (quat q_cum, t) instead of a 3x3 matrix: cheaper compose.
  - Per link, q_link(theta) = E_b * sin(theta/2 + phi_b), b=0..3, with E/phi
    precomputed on host from the link's fixed rotation + joint axis
    (q_link = q_fixed x q_axis(theta) is affine in (sin(t/2), cos(t/2))).
  - q_cum' = q_cum x q_link: 16 scalar_tensor_tensor products (sign and E
    folded into the immediate scalar) + 12 adds.
  - t' = t + rot(q_cum, tf): via a = u x tf + w*tf (constant-coefficient
    combos -> STT chains), b = u x a, t' = t + tf + 2b.
  - Output canonicalization: multiply quat by Sign(w) (Abs for w itself),
    matching the reference's w>=0 convention.

Sharding: pure batch data-parallel across 8 cores (32768 batch each), one
[128 x 256] SBUF megatile per core; the interleaved [128, 256, 7] output
tile doubles as the recurrence state, DMA'd out per link.
"""
import sys
import numpy as np

for _p in ("/opt/trn_rl_repo", "/root/.axon_site/_ro/trn_rl_repo"):
    if _p not in sys.path:
        sys.path.append(_p)

P = 128
L = 32
B_TOTAL = 262144
N_CORES = 8
B_CORE = B_TOTAL // N_CORES      # 32768
F = B_CORE // P                  # 256

# quaternion product q_new_i = sum_k sign * qold_a * p_b   (x,y,z,w = 0..3)
_PROD = [
    (0, 3, 0, +1.0), (0, 0, 3, +1.0), (0, 1, 2, +1.0), (0, 2, 1, -1.0),
    (1, 3, 1, +1.0), (1, 0, 2, -1.0), (1, 1, 3, +1.0), (1, 2, 0, +1.0),
    (2, 3, 2, +1.0), (2, 0, 1, +1.0), (2, 1, 0, -1.0), (2, 2, 3, +1.0),
    (3, 3, 3, +1.0), (3, 0, 0, -1.0), (3, 1, 1, -1.0), (3, 2, 2, -1.0),
]


def _quat_mul(a, b):
    ax, ay, az, aw = a[..., 0], a[..., 1], a[..., 2], a[..., 3]
    bx, by, bz, bw = b[..., 0], b[..., 1], b[..., 2], b[..., 3]
    return np.stack([
        aw * bx + ax * bw + ay * bz - az * by,
        aw * by - ax * bz + ay * bw + az * bx,
        aw * bz + ax * by - ay * bx + az * bw,
        aw * bw - ax * bx - ay * by - az * bz,
    ], axis=-1)


def _mat_to_quat(R):
    """Shepperd largest-pivot matrix->quat (x,y,z,w), float64, per-matrix."""
    out = np.zeros(R.shape[:-2] + (4,), dtype=np.float64)
    for idx in np.ndindex(R.shape[:-2]):
        m = R[idx].astype(np.float64)
        tr = m[0, 0] + m[1, 1] + m[2, 2]
        cand = np.array([1 + tr,
                         1 + m[0, 0] - m[1, 1] - m[2, 2],
                         1 - m[0, 0] + m[1, 1] - m[2, 2],
                         1 - m[0, 0] - m[1, 1] + m[2, 2]])
        p = int(np.argmax(cand))
        s = 0.5 * np.sqrt(cand[p])
        if p == 0:
            w, x = s, (m[2, 1] - m[1, 2]) / (4 * s)
            y, z = (m[0, 2] - m[2, 0]) / (4 * s), (m[1, 0] - m[0, 1]) / (4 * s)
        elif p == 1:
            x, w = s, (m[2, 1] - m[1, 2]) / (4 * s)
            y, z = (m[0, 1] + m[1, 0]) / (4 * s), (m[0, 2] + m[2, 0]) / (4 * s)
        elif p == 2:
            y, w = s, (m[0, 2] - m[2, 0]) / (4 * s)
            x, z = (m[0, 1] + m[1, 0]) / (4 * s), (m[1, 2] + m[2, 1]) / (4 * s)
        else:
            z, w = s, (m[1, 0] - m[0, 1]) / (4 * s)
            x, y = (m[0, 2] + m[2, 0]) / (4 * s), (m[1, 2] + m[2, 1]) / (4 * s)
        if w < 0:
            x, y, z, w = -x, -y, -z, -w
        out[idx] = (x, y, z, w)
    return out


def _build_constants(link_trans, link_rot, joint_axes):
    """Per link, q_link(theta)_b = A_b sin(theta/2) + B_b cos(theta/2).
    The ACT Sin table is only accurate for |arg| < pi, so the device computes
    the basis pair (s2 = sin(u), c2 = sin(pi/2 - |u|) = cos(u)) and forms
    p~_b = base_b + ratio_b * other_b with |ratio| <= 1; the larger coefficient
    C_b is folded into downstream STT immediates (q_link_b = C_b * p~_b).

    Returns C[L,4], ratio[L,4], use_cos_base[L,4] (bool), v[L,3]."""
    qf = _mat_to_quat(np.asarray(link_rot, dtype=np.float64))
    ax = np.asarray(joint_axes, dtype=np.float64)
    axq = np.concatenate([ax, np.zeros((L, 1))], axis=-1)
    A = _quat_mul(qf, axq)        # coefficient of sin(theta/2)
    Bc = qf                       # coefficient of cos(theta/2)
    use_cos = np.abs(Bc) >= np.abs(A)
    C = np.where(use_cos, Bc, A)
    safe = np.where(C == 0.0, 1.0, C)
    ratio = np.where(use_cos, A, Bc) / safe
    ratio = np.where(C == 0.0, 0.0, ratio)
    return C, ratio, use_cos, np.asarray(link_trans, dtype=np.float64)


def _a_chain_consts(v):
    """Per t-component i: sources (u_j, u_k, w) with coefs (v_k, -v_j, v_i).
    Returns for each i: (order of STT ops, base source, cm) with the largest
    |coef| term factored out (its source is the unscaled in1 seed)."""
    plans = []
    for i in range(3):
        j, k = (i + 1) % 3, (i + 2) % 3
        srcs = [3 + j, 3 + k, 6]        # out-tile comp indices (u_j, u_k, w)
        coefs = [v[k], -v[j], v[i]]
        im = int(np.argmax(np.abs(coefs)))
        cm = coefs[im]
        rest = [(srcs[n], coefs[n] / cm) for n in range(3) if n != im]
        plans.append((srcs[im], rest, cm))
    return plans


DEFAULT_CFG = {'x1': 'G', 'x2': 'V', 'qraw': 'V', 'bsub': 'G', 'st': 'G',
               'canon0': 'V', 'canon1': 'G', 'canon2': 'G',
               'bp0': 'G', 'bp1': 'G', 'bp2': 'G', 'bp3': 'G', 'bp4': 'G', 'bp5': 'G'}


def _emit(tc, q_ap, out_aps, C, ratio, use_cos, v64, mybir, cfg=None, reps=1):
    """Emit the per-core Tile program. q_ap: [B_CORE, 32] DRAM; out_aps[l]:
    [B_CORE, 7] DRAM per link."""
    nc = tc.nc
    cfg = cfg or dict(DEFAULT_CFG)
    E = lambda key: {'V': nc.vector, 'G': nc.gpsimd}[cfg[key]]
    f32 = mybir.dt.float32
    Alu = mybir.AluOpType
    Act = mybir.ActivationFunctionType
    from contextlib import ExitStack

    ctx = ExitStack()
    qpool = ctx.enter_context(tc.tile_pool(name="qin", bufs=1))
    outpool = ctx.enter_context(tc.tile_pool(name="out", bufs=3))
    ppool = ctx.enter_context(tc.tile_pool(name="p", bufs=2))
    spool = ctx.enter_context(tc.tile_pool(name="scratch", bufs=2))

    q_sb = qpool.tile([P, F * L], f32, tag="q_sb", name="q_sb")
    nc.sync.dma_start(q_sb[:], q_ap.rearrange("(p t) l -> p (t l)", p=P))
    q3 = q_sb[:].rearrange("p (t l) -> p t l", l=L)

    import contextlib
    loop_ctx = tc.For_i(0, reps, 1) if reps > 1 else contextlib.nullcontext()
    with loop_ctx:
      prev = None   # previous out-tile 3d view
      for l in range(L):
          a_plans = _a_chain_consts(v64[l])
          po = outpool.tile([P, F * 7], f32, tag="po", name="po")
          po3 = po[:].rearrange("p (t c) -> p t c", c=7)

          # --- basis pair s2 = sin(u), c2 = cos(u) = sin(pi/2 - |u|) ------
          au = spool.tile([P, F], f32, tag="au", name="au")
          s2 = ppool.tile([P, F], f32, tag="s2", name="s2")
          c2 = ppool.tile([P, F], f32, tag="c2", name="c2")
          nc.scalar.activation(au[:], q3[:, :, l], Act.Abs)
          nc.scalar.activation(s2[:], q3[:, :, l], Act.Sin)
          nc.scalar.activation(c2[:], au[:], Act.Sin,
                               bias=float(np.pi / 2), scale=-1.0)
          # --- p~_b = base + ratio * other  (q_link_b = C_b * p~_b) -------
          pt = ppool.tile([P, F * 4], f32, tag="pt", name="pt")
          pt3 = pt[:].rearrange("p (t c) -> p t c", c=4)
          for b in range(4):
              base = c2 if use_cos[l][b] else s2
              other = s2 if use_cos[l][b] else c2
              nc.vector.scalar_tensor_tensor(pt3[:, :, b], other[:],
                                             float(ratio[l][b]), base[:],
                                             Alu.mult, Alu.add)

          qraw = spool.tile([P, F * 4], f32, tag="qraw", name="qraw")
          qraw3 = qraw[:].rearrange("p (t c) -> p t c", c=4)

          if l == 0:
              for b in range(4):
                  nc.vector.tensor_scalar_mul(qraw3[:, :, b], pt3[:, :, b],
                                              float(C[0][b]))
              for i in range(3):
                  nc.gpsimd.memset(po3[:, :, i], float(v64[0][i]))
          else:
              # --- compose: 16 STT products into 4 term-group tiles, then
              #     pairwise big adds (scalar_tensor_tensor is DVE-only) ---
              gts = [spool.tile([P, F * 4], f32, tag=f"g{k}", name=f"g{k}") for k in range(4)]
              g3 = [g[:].rearrange("p (t c) -> p t c", c=4) for g in gts]
              for k, (i, a, b, s) in enumerate(_PROD):
                  # term j of output i goes to group tile j at slot i
                  nc.vector.scalar_tensor_tensor(g3[k % 4][:, :, i], pt3[:, :, b],
                                                 float(s * C[l][b]), prev[:, :, 3 + a],
                                                 Alu.mult, Alu.mult)
              x1 = spool.tile([P, F * 4], f32, tag="x1", name="x1")
              x2 = spool.tile([P, F * 4], f32, tag="x2", name="x2")
              E('x1').tensor_add(x1[:], gts[0][:], gts[1][:])
              E('x2').tensor_add(x2[:], gts[2][:], gts[3][:])
              E('qraw').tensor_add(qraw[:], x1[:], x2[:])
              # --- t update ----------------------------------------------
              at = spool.tile([P, F * 3], f32, tag="at", name="at")
              at3 = at[:].rearrange("p (t c) -> p t c", c=3)
              cms = []
              for i in range(3):
                  base, rest, cm = a_plans[i]
                  (s1, c1), (s2, c2) = rest
                  nc.vector.scalar_tensor_tensor(at3[:, :, i], prev[:, :, s1],
                                                 float(c1), prev[:, :, base],
                                                 Alu.mult, Alu.add)
                  nc.vector.scalar_tensor_tensor(at3[:, :, i], prev[:, :, s2],
                                                 float(c2), at3[:, :, i],
                                                 Alu.mult, Alu.add)
                  cms.append(cm)
              asc = spool.tile([P, F * 3], f32, tag="asc", name="asc")
              asc3 = asc[:].rearrange("p (t c) -> p t c", c=3)
              for i in range(3):
                  # asc_i = 2 * cm_i * at_i  (true a-component, x2 folded)
                  nc.scalar.activation(asc3[:, :, i], at3[:, :, i], Act.Copy,
                                       bias=0.0, scale=float(2.0 * cms[i]))
              bm1 = spool.tile([P, F * 3], f32, tag="bm1", name="bm1")
              bm13 = bm1[:].rearrange("p (t c) -> p t c", c=3)
              bm2 = spool.tile([P, F * 3], f32, tag="bm2", name="bm2")
              bm23 = bm2[:].rearrange("p (t c) -> p t c", c=3)
              for i in range(3):
                  j, k = (i + 1) % 3, (i + 2) % 3
                  # b2_i = u_j * (2 a_k) - u_k * (2 a_j)
                  E(f'bp{2*i}').tensor_mul(bm13[:, :, i], asc3[:, :, k],
                                           prev[:, :, 3 + j])
                  E(f'bp{2*i+1}').tensor_mul(bm23[:, :, i], asc3[:, :, j],
                                             prev[:, :, 3 + k])
              b2 = spool.tile([P, F * 3], f32, tag="b2", name="b2")
              E('bsub').tensor_sub(b2[:], bm1[:], bm2[:])
              st = spool.tile([P, F * 3], f32, tag="st", name="st")
              st3 = st[:].rearrange("p (t c) -> p t c", c=3)
              E('st').tensor_add(st3[:, :, :], b2[:].rearrange("p (t c) -> p t c", c=3),
                                  prev[:, :, 0:3])
              for i in range(3):
                  nc.scalar.activation(po3[:, :, i], st3[:, :, i], Act.Copy,
                                       bias=float(v64[l][i]), scale=1.0)

          # --- canonicalize + write quat ---------------------------------
          sg = spool.tile([P, F], f32, tag="sg", name="sg")
          nc.scalar.activation(sg[:], qraw3[:, :, 3], Act.Sign)
          for i in range(3):
              E(f'canon{i}').tensor_mul(po3[:, :, 3 + i], qraw3[:, :, i], sg[:])
          nc.scalar.activation(po3[:, :, 6], qraw3[:, :, 3], Act.Abs)

          nc.sync.dma_start(out_aps[l].rearrange("(p t) c -> p (t c)", p=P), po[:])
          prev = po3
    ctx.close()


def _build_program(C, ratio, use_cos, v64, cfg=None, reps=1):
    import concourse.tile as tile
    from concourse import bacc, mybir

    nc = bacc.Bacc("TRN2", target_bir_lowering=False, debug=False,
                   enable_asserts=False, num_devices=N_CORES)
    f32 = mybir.dt.float32

    # non-Copy activation float biases require pre-registered const APs
    for val in (float(np.pi / 2),):
        if (f32, val) not in nc.const_aps.aps:
            t = nc.alloc_sbuf_tensor(f"const-f32-{val}", [128, 1], f32)
            nc.gpsimd.memset(t.ap(), val)
            nc.const_aps.aps[(f32, val)] = t.ap()
    nc.all_engine_barrier()
    q_ap = nc.dram_tensor("q", [B_CORE, L], f32, kind="ExternalInput").ap()
    out_aps = [nc.dram_tensor(f"out{l}", [B_CORE, 7], f32, kind="ExternalOutput").ap()
               for l in range(L)]
    with tile.TileContext(nc) as tc:
        _emit(tc, q_ap, out_aps, C, ratio, use_cos, v64, mybir, cfg=cfg, reps=reps)
    nc.compile()
    return nc


TRACE = False      # set True (e.g. from test.py) to NTFF-profile the run
LAST = None        # BassKernelResults of the most recent kernel() call


def kernel(q, link_trans, link_rot, joint_axes):
    from concourse.bass_utils import run_bass_kernel_spmd

    C, ratio, use_cos, v64 = _build_constants(link_trans, link_rot, joint_axes)
    nc = _build_program(C, ratio, use_cos, v64)

    # host-side half-angle wrap keeps every ACT Sin argument within +-3pi/2
    qh = np.asarray(q, dtype=np.float32) * np.float32(0.5)
    qh = (qh + np.float32(np.pi)) % np.float32(2 * np.pi) - np.float32(np.pi)
    in_maps = [{"q": np.ascontiguousarray(qh[c * B_CORE:(c + 1) * B_CORE])}
               for c in range(N_CORES)]
    import time
    t0 = time.time()
    res = run_bass_kernel_spmd(nc, in_maps, list(range(N_CORES)))
    exec1 = time.time() - t0
    global LAST, EXEC_WALL_S
    LAST = res
    EXEC_WALL_S = exec1
    if TRACE:
        # warm second execution for a dispatch+exec wall-clock measurement
        t0 = time.time()
        res = run_bass_kernel_spmd(nc, in_maps, list(range(N_CORES)))
        EXEC_WALL_S = time.time() - t0
        LAST = res
    per_core = [np.stack([r[f"out{l}"] for l in range(L)], axis=0)
                for r in res.results]
    return np.concatenate(per_core, axis=1)

